# revision 1
# baseline (speedup 1.0000x reference)
"""Distributed Trainium2 attention kernel (8 NeuronCores).

Sharding: 4-way data parallel over batch x 2-way tensor parallel over heads.
Core c handles batch c//2 and head-group c%2 (8 of 16 heads). Host sums the
two row-parallel out-proj partials per batch.

Key perf structure (vs the earlier baseline):
- Single activation table (natural_log_exp): rms-norm uses exp(-0.5*ln(ms))
  instead of Sqrt, so the softmax Exp never triggers a mid-kernel
  ACT_TABLE_LOAD. Any >=2us PE stall risks a 655us HAM 4/8 down-clock window.
- Softmax exp split: columns [0:EXPA) on the Act engine (true Exp),
  [EXPA:1024) on the otherwise-idle DVE via a one-op Schraudolph fast-exp
  (fused y=A*x+B with int16 convert, bitcast to bf16; the bf16 staircase
  matches the Act path's bf16 output). Global scale bias cancels in softmax.
- Phase order: k/v projections for all 32 t-tiles, then q tiles 0-7, then
  attention chunks; q tiles 8-31 are processed inside the chunk loop (DMA,
  projection, rotary, transpose pipelined across head iterations) so their
  DVE work hides under attention matmuls.
- Softmax denominators ride as psum row 64 (ones column in vaug); normalize
  uses reciprocal_approx_fast (1 DVE op) on the gpsimd-broadcast row.
- PE warm-up bursts bridge the two spots where the PE would otherwise idle
  >1us (phase-2 entry DVE lag, attention->out-proj drain): idle windows
  trigger the HAM clock gate (observed 655.36us at half PE throughput).
"""
import sys
import os
from contextlib import ExitStack

if '/opt/trn_rl_repo' not in sys.path:
    sys.path.insert(0, '/opt/trn_rl_repo')

import numpy as np
import ml_dtypes

bf16 = ml_dtypes.bfloat16

T = 4096
D = 1024
HL = 8          # local heads per core
HD = 64
NT = T // 128   # 32 t-tiles
KT = D // 128   # 8 contraction tiles for projections
NCH = 4         # chunks of 1024 along t for attention
CW = 1024       # chunk width
PAIRS = 4       # head pairs per core
EPS = 1.1920928955078125e-07

EXPA = 1024     # softmax cols per 1024 on Act engine (rest: DVE fast-exp).
                # 1024 = Act-paced phase C: ~5% slower steady-state than a
                # DVE split, but z never depends on the DVE queue, which
                # keeps PE stalls under the HAM down-clock threshold.
                # Chunk 3 hides chunks 0-2's out-projection in PE idle.
LN2 = 0.6931471805599453
# z = bitcast_bf16(int16(psc * SCHRA + SCHRB)) ~= exp(0.125 * psc) * const
SCHRA = 0.125 * (2.0 ** 23 / LN2) / 65536.0
SCHRB = (127.0 * 2.0 ** 23 - 485000.0) / 65536.0

# interleaved q-tile pipeline schedule (relative tile 0..7 within next
# chunk): DMA one head ahead; projection + rotary dripped in small stages
# across the head's s-loop so no DVE burst can stall the z pipeline
DMA_SCHED = {0: [0, 1, 2], 1: [3], 2: [4], 3: [5], 4: [6], 5: [7]}
PROJ3 = {
    1: [(0, 4, (8, 12, 16, 20)), (1, 6, (10, 14, 18, 22))],
    2: [(2, 6, (10, 14, 18, 22))],
    3: [(3, 6, (10, 14, 18, 22))],
    4: [(4, 6, (10, 14, 18, 22))],
    5: [(5, 6, (10, 14, 18, 22))],
    6: [(6, 4, (8, 12, 16, 20)), (7, 6, (10, 14, 18, 22))],
}
PO_SLOTS = (6, 10, 14, 18, 22, 26)   # chunk-3 s-slots carrying one po each


def build():
    from concourse import bacc, tile, mybir

    BF16 = mybir.dt.bfloat16
    F32 = mybir.dt.float32
    I16 = mybir.dt.int16
    I32 = mybir.dt.int32
    AF = mybir.ActivationFunctionType
    ALU = mybir.AluOpType
    AX = mybir.AxisListType

    nc = bacc.Bacc()
    xT = nc.declare_dram_parameter("xT", [D, T], BF16, isOutput=False)
    wqT = nc.declare_dram_parameter("wqT", [D, 512], BF16, isOutput=False)
    wkT = nc.declare_dram_parameter("wkT", [D, 512], BF16, isOutput=False)
    wvT = nc.declare_dram_parameter("wvT", [D, 512], BF16, isOutput=False)
    woT = nc.declare_dram_parameter("woT", [512, D], BF16, isOutput=False)
    cos2 = nc.declare_dram_parameter("cos2", [T, 64], BF16, isOutput=False)
    ss = nc.declare_dram_parameter("ss", [T, 64], BF16, isOutput=False)
    ident = nc.declare_dram_parameter("ident", [128, 128], BF16, isOutput=False)
    out = nc.declare_dram_parameter("out", [T, D], F32, isOutput=True)

    with tile.TileContext(nc) as tc:
        with tc.tile_pool(name="persist", bufs=1) as persist:
            qTc = [persist.tile([128, PAIRS, CW], BF16, tag=f"qT{c}",
                                name=f"qT{c}") for c in range(NCH)]
            kT = persist.tile([128, PAIRS, T], BF16, tag="kT")
            vaug = persist.tile([128, NT, HL, 65], BF16, tag="vaug")
            wo_sb = persist.tile([128, 4, D], BF16, tag="wo_sb")
            id_sb = persist.tile([128, 128], BF16, tag="id_sb")
            eps_t = persist.tile([128, 1], F32, tag="eps_t")
            yTn = persist.tile([128, PAIRS, T], BF16, tag="yTn")

            nc.vector.memset(vaug[:, :, :, 64:65], 1.0)
            nc.vector.memset(eps_t[:], EPS)

            # pools alive through phases A..C
            span = ExitStack()
            with span:
                wqp = span.enter_context(tc.tile_pool(name="wqp", bufs=1))
                xcolp = span.enter_context(tc.tile_pool(name="xcolp", bufs=3))
                cscr = span.enter_context(tc.tile_pool(name="cscr", bufs=2))
                csmall = span.enter_context(tc.tile_pool(name="csmall", bufs=2))
                ps_tr = span.enter_context(
                    tc.tile_pool(name="ps_tr", bufs=1, space="PSUM"))

                w_sb = {}
                w_sb["q"] = wqp.tile([128, KT, 512], BF16, tag="wq",
                                     name="w_q_sb")
                for ki in range(KT):
                    nc.sync.dma_start(
                        w_sb["q"][:, ki, :], wqT[ki * 128:(ki + 1) * 128, :])
                cos_sb = wqp.tile([128, NT, 64], BF16, tag="cos_sb")
                ss_sb = wqp.tile([128, NT, 64], BF16, tag="ss_sb")
                nc.sync.dma_start(
                    cos_sb[:], cos2[:].rearrange("(t p) d -> p t d", p=128))
                nc.sync.dma_start(
                    ss_sb[:], ss[:].rearrange("(t p) d -> p t d", p=128))
                nc.sync.dma_start(id_sb[:], ident[:])
                nc.sync.dma_start(
                    wo_sb[:], woT[:].rearrange("(k p) n -> p k n", p=128))

                def rotary_rms(t, ps_q, store, tcol, pool_s, pool_m, on_act):
                    """rotary + rms-normalize one projected [128,512] tile;
                    returns the qn tile to transpose later.
                    on_act: phase A/B — copies + Sqrt on Act (sqrt table era).
                    Phase C — everything off-Act (Exp table must stay loaded):
                    copies on DVE, rsqrt via bit-trick + 2 Newton steps.
                    """
                    cp = nc.scalar.copy if on_act else nc.vector.tensor_copy
                    ctb = cos_sb[:, t, :].unsqueeze(1).broadcast_to(
                        [128, HL, 64])
                    stb = ss_sb[:, t, :].unsqueeze(1).broadcast_to(
                        [128, HL, 64])
                    qb = pool_s.tile([128, 512], BF16, tag="qb")
                    cp(qb[:], ps_q[:])
                    b3 = qb[:].rearrange("p (h u d) -> p h u d", h=HL, u=2)
                    qs = pool_s.tile([128, 512], BF16, tag="qs")
                    qs3 = qs[:].rearrange("p (h u d) -> p h u d", h=HL, u=2)
                    cp(qs3[:, :, 0, :], b3[:, :, 1, :])
                    cp(qs3[:, :, 1, :], b3[:, :, 0, :])
                    t1 = pool_s.tile([128, 512], BF16, tag="t1")
                    nc.vector.tensor_tensor(
                        t1[:].rearrange("p (h d) -> p h d", h=HL),
                        qb[:].rearrange("p (h d) -> p h d", h=HL),
                        ctb, op=ALU.mult)
                    r = pool_s.tile([128, 512], BF16, tag="r")
                    nc.vector.tensor_tensor(
                        r[:].rearrange("p (h d) -> p h d", h=HL),
                        qs[:].rearrange("p (h d) -> p h d", h=HL),
                        stb, op=ALU.mult)
                    nc.vector.tensor_tensor(r[:], t1[:], r[:], op=ALU.add)
                    sq = pool_s.tile([128, 512], BF16, tag="sq")
                    # keep GpSimd single-purpose (PartitionBroadcast) --
                    # switching Q7 libraries costs a multi-us reload
                    if on_act:
                        nc.scalar.square(sq[:], r[:])
                    else:
                        nc.vector.tensor_tensor(sq[:], r[:], r[:],
                                                op=ALU.mult)
                    ms8 = pool_m.tile([128, HL], F32, tag="ms8")
                    nc.vector.tensor_reduce(
                        ms8[:], sq[:].rearrange("p (h d) -> p h d", h=HL),
                        axis=AX.X, op=ALU.add)
                    rinv = pool_m.tile([128, HL], F32, tag="rinv")
                    if on_act:
                        rms = pool_m.tile([128, HL], F32, tag="rms")
                        nc.scalar.activation(
                            rms[:], ms8[:], AF.Sqrt, scale=1.0 / HD,
                            bias=eps_t[:])
                        nc.vector.reciprocal(rinv[:], rms[:])
                    else:
                        # y = rsqrt(ms/64 + eps) without the Act engine
                        hd_t = pool_m.tile([128, HL], F32, tag="hd")
                        nc.vector.tensor_scalar(
                            hd_t[:], ms8[:], 0.5 / HD, 0.5 * EPS,
                            ALU.mult, ALU.add)
                        d2 = pool_m.tile([128, HL], F32, tag="d2")
                        nc.vector.tensor_scalar(
                            d2[:], hd_t[:], 2.0, None, ALU.mult)
                        ish = pool_m.tile([128, HL], I32, tag="ish")
                        nc.vector.tensor_scalar(
                            ish[:], d2[:].bitcast(I32), 1, None,
                            ALU.logical_shift_right)
                        y0i = pool_m.tile([128, HL], I32, tag="y0i")
                        nc.vector.tensor_scalar(
                            y0i[:], ish[:], -1, 0x5F3759DF,
                            ALU.mult, ALU.add)
                        y = y0i[:].bitcast(F32)
                        for it in range(2):
                            ysq = pool_m.tile([128, HL], F32,
                                              tag=f"ysq{it}")
                            nc.vector.tensor_tensor(
                                ysq[:], y, y, op=ALU.mult)
                            nc.vector.scalar_tensor_tensor(
                                ysq[:], hd_t[:], -1.0, ysq[:],
                                op0=ALU.mult, op1=ALU.mult)
                            nc.vector.tensor_scalar(
                                ysq[:], ysq[:], 1.5, None, ALU.add)
                            yn = pool_m.tile([128, HL], F32,
                                             tag=f"yn{it}")
                            nc.vector.tensor_tensor(
                                yn[:], y, ysq[:], op=ALU.mult)
                            y = yn[:]
                        nc.vector.tensor_copy(rinv[:], y)
                    qn = pool_s.tile([128, 512], BF16, tag="qn")
                    nc.vector.tensor_tensor(
                        qn[:].rearrange("p (h d) -> p h d", h=HL),
                        r[:].rearrange("p (h d) -> p h d", h=HL),
                        rinv[:].unsqueeze(2).broadcast_to([128, HL, 64]),
                        op=ALU.mult)
                    return (qn, store, tcol)

                def rotary_stages(t, ps_q, store, tcol, pool_s, pool_m,
                                  pend_tr):
                    """Phase-C rotary+rms for one q tile, split into four
                    small DVE emissions (<=1.8us each) dripped across the
                    s-loop; nothing touches Act or GpSimd. The final stage
                    appends the transpose job to pend_tr."""
                    st = {}

                    def st0():
                        qb = pool_s.tile([128, 512], BF16, tag="qb",
                                         name="qb")
                        nc.vector.tensor_copy(qb[:], ps_q[:])
                        qs = pool_s.tile([128, 512], BF16, tag="qs",
                                         name="qs")
                        b3 = qb[:].rearrange("p (h u d) -> p h u d",
                                             h=HL, u=2)
                        qs3 = qs[:].rearrange("p (h u d) -> p h u d",
                                              h=HL, u=2)
                        nc.vector.tensor_copy(qs3[:, :, 0, :], b3[:, :, 1, :])
                        nc.vector.tensor_copy(qs3[:, :, 1, :], b3[:, :, 0, :])
                        st["qb"], st["qs"] = qb, qs

                    def st1():
                        ctb = cos_sb[:, t % NT, :].unsqueeze(1).broadcast_to(
                            [128, HL, 64])
                        stb = ss_sb[:, t % NT, :].unsqueeze(1).broadcast_to(
                            [128, HL, 64])
                        t1 = pool_s.tile([128, 512], BF16, tag="t1",
                                         name="t1")
                        nc.vector.tensor_tensor(
                            t1[:].rearrange("p (h d) -> p h d", h=HL),
                            st["qb"][:].rearrange("p (h d) -> p h d", h=HL),
                            ctb, op=ALU.mult)
                        r = pool_s.tile([128, 512], BF16, tag="r", name="r")
                        nc.vector.tensor_tensor(
                            r[:].rearrange("p (h d) -> p h d", h=HL),
                            st["qs"][:].rearrange("p (h d) -> p h d", h=HL),
                            stb, op=ALU.mult)
                        nc.vector.tensor_tensor(r[:], t1[:], r[:],
                                                op=ALU.add)
                        st["r"] = r

                    def st2():
                        sq = pool_s.tile([128, 512], BF16, tag="sq",
                                         name="sq")
                        nc.vector.tensor_tensor(sq[:], st["r"][:],
                                                st["r"][:], op=ALU.mult)
                        ms8 = pool_m.tile([128, HL], F32, tag="ms8",
                                          name="ms8")
                        nc.vector.tensor_reduce(
                            ms8[:], sq[:].rearrange("p (h d) -> p h d",
                                                    h=HL),
                            axis=AX.X, op=ALU.add)
                        st["ms8"] = ms8

                    def st3():
                        hd_t = pool_m.tile([128, HL], F32, tag="hd",
                                           name="hd_t")
                        nc.vector.tensor_scalar(
                            hd_t[:], st["ms8"][:], 0.5 / HD, 0.5 * EPS,
                            ALU.mult, ALU.add)
                        d2 = pool_m.tile([128, HL], F32, tag="d2", name="d2")
                        nc.vector.tensor_scalar(
                            d2[:], hd_t[:], 2.0, None, ALU.mult)
                        ish = pool_m.tile([128, HL], I32, tag="ish",
                                          name="ish")
                        nc.vector.tensor_scalar(
                            ish[:], d2[:].bitcast(I32), 1, None,
                            ALU.logical_shift_right)
                        y0i = pool_m.tile([128, HL], I32, tag="y0i",
                                          name="y0i")
                        nc.vector.tensor_scalar(
                            y0i[:], ish[:], -1, 0x5F3759DF,
                            ALU.mult, ALU.add)
                        y = y0i[:].bitcast(F32)
                        for it in range(2):
                            ysq = pool_m.tile([128, HL], F32,
                                              tag=f"ysq{it}",
                                              name=f"ysq{it}")
                            nc.vector.tensor_tensor(
                                ysq[:], y, y, op=ALU.mult)
                            nc.vector.scalar_tensor_tensor(
                                ysq[:], hd_t[:], -1.0, ysq[:],
                                op0=ALU.mult, op1=ALU.mult)
                            nc.vector.tensor_scalar(
                                ysq[:], ysq[:], 1.5, None, ALU.add)
                            yn = pool_m.tile([128, HL], F32,
                                             tag=f"yn{it}", name=f"yn{it}")
                            nc.vector.tensor_tensor(
                                yn[:], y, ysq[:], op=ALU.mult)
                            y = yn[:]
                        qn = pool_s.tile([128, 512], BF16, tag="qn",
                                         name="qn")
                        nc.vector.tensor_tensor(
                            qn[:].rearrange("p (h d) -> p h d", h=HL),
                            st["r"][:].rearrange("p (h d) -> p h d", h=HL),
                            y.unsqueeze(2).broadcast_to([128, HL, 64]),
                            op=ALU.mult)
                        pend_tr.append((qn, store, tcol))

                    return [st0, st1, st2, st3]

                def emit_tr(pend):
                    qn, store, tcol = pend
                    tp = ps_tr.tile([128, 4, 128], BF16, tag="tp")
                    for cb in range(4):
                        nc.tensor.transpose(
                            tp[:, cb, :], qn[:, cb * 128:(cb + 1) * 128],
                            id_sb[:])
                    nc.vector.tensor_copy(
                        store[:, :, tcol:tcol + 128], tp[:])

                def dma_xcol(t):
                    xcol = xcolp.tile([128, KT, 128], BF16, tag="xcol")
                    nc.sync.dma_start(
                        xcol[:],
                        xT[:, t * 128:(t + 1) * 128].rearrange(
                            "(k p) t -> p k t", p=128))
                    return xcol

                def proj(xcol, name, ps_pool):
                    ps = ps_pool.tile([128, 512], F32, tag="pqkv",
                                      name=f"ps_{name}")
                    for ki in range(KT):
                        nc.tensor.matmul(
                            ps[:], xcol[:, ki, :], w_sb[name][:, ki, :],
                            start=(ki == 0), stop=(ki == KT - 1))
                    return ps

                # ================= Phase A: k/v for all t =================
                phaseA = ExitStack()
                with phaseA:
                    wkv = phaseA.enter_context(
                        tc.tile_pool(name="wkv", bufs=1))
                    ascr = phaseA.enter_context(
                        tc.tile_pool(name="ascr", bufs=3))
                    asmall = phaseA.enter_context(
                        tc.tile_pool(name="asmall", bufs=3))
                    ps_qkv = phaseA.enter_context(
                        tc.tile_pool(name="ps_qkv", bufs=6, space="PSUM"))

                    for name, param in (("k", wkT), ("v", wvT)):
                        w_sb[name] = wkv.tile([128, KT, 512], BF16,
                                              tag=f"w{name}",
                                              name=f"w_{name}_sb")
                        for ki in range(KT):
                            nc.sync.dma_start(
                                w_sb[name][:, ki, :],
                                param[ki * 128:(ki + 1) * 128, :])

                    # preload the GpSimd PartitionBroadcast library now so
                    # phase C's first broadcast doesn't eat a Q7 reload
                    pbsrc = csmall.tile([1, 8], F32, tag="pbsrc")
                    nc.vector.memset(pbsrc[:], 1.0)
                    pbdst = csmall.tile([128, 8], F32, tag="pbdst")
                    nc.gpsimd.partition_broadcast(pbdst[:], pbsrc[:])

                    pendq = []  # transposes pending, emitted with lag 2
                    for t in range(NT):
                        xcol = dma_xcol(t)
                        ps_k = proj(xcol, "k", ps_qkv)
                        ps_v = proj(xcol, "v", ps_qkv)
                        nc.scalar.copy(
                            vaug[:, t, :, 0:64],
                            ps_v[:].rearrange("p (h d) -> p h d", h=HL))
                        if len(pendq) >= 2:
                            emit_tr(pendq.pop(0))
                        pendq.append(rotary_rms(t, ps_k, kT, t * 128,
                                                ascr, asmall, on_act=True))

                    # ============= Phase B: q tiles 0..7 =============
                    # PE work per q tile (~2.1us) undercuts the rotary chain
                    # (~3us): pad each step with a small PE burst so the PE
                    # never idles waiting on the psum ring (HAM clock gate)
                    for t in range(8):
                        xcol = dma_xcol(t)
                        ps_q = proj(xcol, "q", ps_qkv)
                        wupb = ps_qkv.tile([128, 512], F32, tag="pqkv",
                                           name=f"wupb{t}")
                        for i in range(6):
                            nc.tensor.matmul(
                                wupb[:], kT[0:64, 0, 0:128],
                                kT[0:64, 0, 0:512],
                                start=(i == 0), stop=(i == 5))
                        if len(pendq) >= 2:
                            emit_tr(pendq.pop(0))
                        pendq.append(rotary_rms(t, ps_q, qTc[0],
                                                (t % 8) * 128,
                                                ascr, asmall, on_act=True))
                    emit_tr(pendq.pop(0))
                    # preload the Exp table (ATL hides under the burst) and
                    # bridge the DVE lag on the last q tile's rotary with a
                    # PE warm-up burst: no PE idle into the first scores
                    # matmul (HAM clock gate)
                    dummy = csmall.tile([128, 1], F32, tag="dummy")
                    nc.scalar.activation(dummy[:], eps_t[:], AF.Exp)
                    wup = ps_qkv.tile([128, 512], F32, tag="pqkv",
                                      name="wup")
                    for i in range(64):
                        nc.tensor.matmul(
                            wup[:], kT[0:64, 0, T - 128:T],
                            kT[0:64, 0, 0:512],
                            start=(i == 0), stop=(i == 63))
                    emit_tr(pendq.pop(0))
                    assert not pendq

                # ================= Phase C: attention =================
                with (
                    tc.tile_pool(name="zp2", bufs=4) as zp2,
                    tc.tile_pool(name="nrm", bufs=2) as nrm,
                    tc.tile_pool(name="nrm1", bufs=1) as nrm1,
                    tc.tile_pool(name="ostg", bufs=3) as ostg,
                    tc.tile_pool(name="ps_sc", bufs=2, space="PSUM") as ps_sc,
                    tc.tile_pool(name="ps_y", bufs=1, space="PSUM") as ps_y,
                ):
                    def emit_po(ch_po, tt, oc):
                        tsl = slice(ch_po * CW + tt * 128,
                                    ch_po * CW + (tt + 1) * 128)
                        po = ps_tr.tile([128, 512], F32, tag="pqkv",
                                        name="po")
                        for kp in range(4):
                            nc.tensor.matmul(
                                po[:], yTn[:, kp, tsl],
                                wo_sb[:, kp, oc * 512:(oc + 1) * 512],
                                start=(kp == 0), stop=(kp == 3))
                        ost = ostg.tile([128, 512], F32, tag="ost")
                        nc.vector.tensor_copy(ost[:], po[:])
                        nc.sync.dma_start(
                            out[tsl, oc * 512:(oc + 1) * 512], ost[:])

                    # chunks 0-2's out-projections ride inside chunk 3's
                    # Act-paced head iterations (PE has idle there)
                    po_units = [(cp, tt, oc) for cp in range(NCH - 1)
                                for tt in range(8) for oc in range(2)]
                    pend_tr = []    # q tiles awaiting transpose
                    pend_proj = {}  # rel -> xcol DMA'd, awaiting projection
                    pend_norm_a = []  # deferred denom extract+broadcast
                    pend_norm_b = []  # deferred reciprocal+scale
                    for ch in range(NCH):
                        chs = slice(ch * CW, (ch + 1) * CW)
                        expa = CW if ch == NCH - 1 else EXPA
                        for h in range(HL):
                            rsl = slice((h % 2) * 64, (h % 2) * 64 + 64)
                            pr = h // 2
                            stage_ev = {}
                            if ch < NCH - 1:
                                base = 8 * (ch + 1)
                                for rel, s_proj, s_stages in PROJ3.get(h, []):
                                    stage_ev[s_proj] = (rel, s_stages)
                            ya = ps_y.tile([65, CW], F32, tag="ya")
                            for s in range(NT):
                                # previous head's normalize + q-tile work,
                                # staged in small steps across this head's
                                # s-loop so no single DVE/Pool burst can
                                # back up the z queue and stall attnV
                                if s == 2:
                                    for fn in pend_norm_a:
                                        fn()
                                    pend_norm_a = []
                                    for pend in pend_tr:
                                        emit_tr(pend)
                                    pend_tr = []
                                if s in (3, 5, 7, 9) and pend_norm_b:
                                    pend_norm_b[0]((s - 3) // 2)
                                    if s == 9:
                                        pend_norm_b = []
                                if s in stage_ev:
                                    ev = stage_ev.pop(s)
                                    if isinstance(ev, tuple):
                                        rel, s_stages = ev
                                        ps_q = proj(pend_proj.pop(rel),
                                                    "q", ps_tr)
                                        stages = rotary_stages(
                                            base + rel, ps_q, qTc[ch + 1],
                                            rel * 128, cscr, csmall,
                                            pend_tr)
                                        for ss, fn in zip(s_stages, stages):
                                            stage_ev[ss] = fn
                                    else:
                                        ev()
                                if s == 26 and ch < NCH - 1:
                                    for rel in DMA_SCHED.get(h, []):
                                        pend_proj[rel] = dma_xcol(
                                            8 * (ch + 1) + rel)
                                if ch == NCH - 1 and s in PO_SLOTS \
                                        and po_units:
                                    emit_po(*po_units.pop(0))
                                ssl = slice(s * 128, (s + 1) * 128)
                                psc = ps_sc.tile([128, CW], F32, tag="psc")
                                # halves in 1,0 order: z for half1 is pure
                                # Act and consumed first, so the DVE
                                # fast-exp (first columns, feeding half0)
                                # gets ~1 s-iter of latency slack
                                for half in (1, 0):
                                    hsl = slice(half * 512, (half + 1) * 512)
                                    nc.tensor.matmul(
                                        psc[:, hsl], kT[rsl, pr, ssl],
                                        qTc[ch][rsl, pr, hsl],
                                        start=True, stop=True,
                                        tile_position=((h % 2) * 64, 0))
                                z = zp2.tile([128, CW], BF16, tag="z")
                                nc.scalar.activation(
                                    z[:, 0:expa], psc[:, 0:expa], AF.Exp,
                                    scale=0.125)
                                if expa < CW:
                                    nc.vector.tensor_scalar(
                                        z[:, expa:CW].bitcast(I16),
                                        psc[:, expa:CW],
                                        SCHRA, SCHRB, ALU.mult, ALU.add)
                                if s == 0:
                                    # head-boundary filler: the first z is
                                    # ~1.3us behind the PE; chew on junk so
                                    # the PE never idles (HAM clock gate)
                                    jb = ps_tr.tile([128, 512], F32,
                                                    tag="pqkv", name="jb")
                                    for i in range(4):
                                        nc.tensor.matmul(
                                            jb[:], kT[0:64, 0, 0:128],
                                            kT[0:64, 0, 0:512],
                                            start=(i == 0), stop=(i == 3))
                                for half in (1, 0):
                                    hsl = slice(half * 512, (half + 1) * 512)
                                    nc.tensor.matmul(
                                        ya[:, hsl], vaug[:, s, h, :],
                                        z[:, hsl],
                                        start=(s == 0), stop=(s == NT - 1))
                            # evacuate psum + broadcast denom row now (frees
                            # the ya bank); defer reciprocal+scale into the
                            # next head's s-loop
                            yu = nrm.tile([65, CW], F32, tag="yu")
                            nc.vector.tensor_copy(yu[:], ya[:])
                            st = {"yu": yu, "rsl": rsl, "pr": pr, "chs": chs}

                            def norm_a(st=st):
                                dtmp = nrm1.tile([1, CW], F32, tag="dtmp",
                                                 name="dtmp")
                                nc.vector.tensor_copy(
                                    dtmp[:], st["yu"][64:65, :])
                                bc = nrm1.tile([128, CW], F32, tag="bc",
                                               name="bc")
                                nc.gpsimd.partition_broadcast(bc[:], dtmp[:])
                                st["bc"] = bc

                            def norm_b(step, st=st):
                                if step == 0:
                                    st["bcr"] = nrm1.tile(
                                        [128, CW], F32, tag="bcr",
                                        name="bcr")
                                    nc.vector.reciprocal_approx_fast(
                                        st["bcr"][:, 0:512],
                                        st["bc"][:, 0:512])
                                elif step == 1:
                                    nc.vector.reciprocal_approx_fast(
                                        st["bcr"][:, 512:CW],
                                        st["bc"][:, 512:CW])
                                else:
                                    hs = slice(0, 512) if step == 2 \
                                        else slice(512, CW)
                                    cs0 = st["chs"].start
                                    nc.vector.tensor_tensor(
                                        yTn[st["rsl"], st["pr"],
                                            cs0 + hs.start:cs0 + hs.stop],
                                        st["yu"][0:64, hs],
                                        st["bcr"][0:64, hs], op=ALU.mult)

                            pend_norm_a.append(norm_a)
                            pend_norm_b.append(norm_b)
                        assert not pend_proj, (ch, pend_proj)
                    for fn in pend_norm_a:
                        fn()
                    for fn in pend_norm_b:
                        for step in range(4):
                            fn(step)

                # ===== tail: chunk 3's own out-projection =====
                with (
                    tc.tile_pool(name="ps_po", bufs=2, space="PSUM") as ps_po,
                    tc.tile_pool(name="ostg2", bufs=3) as ostg2,
                ):
                    # bridge burst across the last head's normalize drain
                    wdn = ps_po.tile([128, 512], F32, tag="po", name="wdn")
                    for i in range(56):
                        nc.tensor.matmul(
                            wdn[:], kT[0:64, 0, T - 128:T],
                            kT[0:64, 0, 0:512],
                            start=(i == 0), stop=(i == 55))
                    for tt in range(8):
                        tsl = slice((NCH - 1) * CW + tt * 128,
                                    (NCH - 1) * CW + (tt + 1) * 128)
                        for oc in range(2):
                            po = ps_po.tile([128, 512], F32, tag="po")
                            for kp in range(4):
                                nc.tensor.matmul(
                                    po[:], yTn[:, kp, tsl],
                                    wo_sb[:, kp, oc * 512:(oc + 1) * 512],
                                    start=(kp == 0), stop=(kp == 3))
                            ost = ostg2.tile([128, 512], F32, tag="ost")
                            nc.vector.tensor_copy(ost[:], po[:])
                            nc.sync.dma_start(
                                out[tsl, oc * 512:(oc + 1) * 512], ost[:])

    nc.compile()
    return nc


_CACHE = {}


def _get_nc():
    if "nc" not in _CACHE:
        _CACHE["nc"] = build()
    return _CACHE["nc"]


def _prep_inputs(x, cos, sin, wq, wk, wv, wo):
    x = np.asarray(x, dtype=np.float32)
    cos = np.asarray(cos, dtype=np.float32).reshape(T, 32)
    sin = np.asarray(sin, dtype=np.float32).reshape(T, 32)
    wq = np.asarray(wq, dtype=np.float32)
    wk = np.asarray(wk, dtype=np.float32)
    wv = np.asarray(wv, dtype=np.float32)
    wo = np.asarray(wo, dtype=np.float32)

    cos2 = np.concatenate([cos, cos], axis=1)
    ss = np.concatenate([sin, -sin], axis=1)
    ident = np.eye(128, dtype=bf16)

    in_maps = []
    for c in range(8):
        b, hg = c // 2, c % 2
        rows = slice(hg * 512, (hg + 1) * 512)
        in_maps.append({
            "xT": np.ascontiguousarray(x[b].T).astype(bf16),
            "wqT": np.ascontiguousarray(wq[rows, :].T).astype(bf16),
            "wkT": np.ascontiguousarray(wk[rows, :].T).astype(bf16),
            "wvT": np.ascontiguousarray(wv[rows, :].T).astype(bf16),
            "woT": np.ascontiguousarray(wo[:, rows].T).astype(bf16),
            "cos2": cos2.astype(bf16),
            "ss": ss.astype(bf16),
            "ident": ident,
        })
    return in_maps


def _run(in_maps, trace=False):
    from concourse.bass_utils import run_bass_kernel_spmd

    nc = _get_nc()
    res = run_bass_kernel_spmd(nc, in_maps, core_ids=list(range(8)),
                               trace=trace)
    parts = [res.results[c]["out"] for c in range(8)]
    full = np.stack([parts[2 * b] + parts[2 * b + 1] for b in range(4)])
    return full.astype(np.float32), res


def kernel(x, cos, sin, wq, wk, wv, wo):
    in_maps = _prep_inputs(x, cos, sin, wq, wk, wv, wo)
    full, _ = _run(in_maps, trace=False)
    return full



# revision 2
# speedup vs baseline: 1.0733x; 1.0733x over previous
"""Distributed Trainium2 attention kernel (8 NeuronCores).

Sharding: 4-way data parallel over batch x 2-way tensor parallel over heads.
Core c handles batch c//2 and head-group c%2 (8 of 16 heads). Host sums the
two row-parallel out-proj partials per batch.

Structure (v2 — head-pair row-tiled attention):
- Phase A: per t-tile, all three projections (q,k,v) + rotary+rms for q and
  k + PE transposes into kT/qTc. PE ~5.7us/tile paces; Act does the copies/
  square/sqrt, DVE the rotary mults/reduce/reciprocal/scale.
- Phase C: heads processed in PAIRS. The two K=64 scores matmuls of a pair
  run CONCURRENTLY in the PE array via row tiling (tile_position (0,0) and
  (64,0)) — kT/qTc store the pair split at partition 64, so both tiles
  stream complementary partition ranges of the same SBUF columns (row
  tiling uses no extra XBUS). Per (pair, s-tile): 512 cy scores + 2x512 cy
  attnV = 1536 cy vs 4096 in the per-head serial schedule.
- Softmax exp split to stay under the PE pace (640ns/step): one big Act
  instruction (cols [0:EXPA] of the pair's [128,1024] psc) + one DVE
  Schraudolph fast-exp (cols [EXPA:1024]). Per-query exp path is constant
  across s so the approximation partially cancels in softmax.
- attnV lags scores by 3 steps (z double-buffered 4x); psc double-buffered
  (2x2 banks), ya pair double-buffered (2x2 banks) = 8 PSUM banks exactly.
- Softmax denominators ride as psum row 64 (ones column in vaug); per-pair
  normalize staged in small closures across the next pair's s-loop.
- Out-projection as a tail loop (PE has no idle in phase C); early po units
  only need early chunks' yTn so the final normalize drain hides under it.
"""
import sys
import os
from contextlib import ExitStack

if '/opt/trn_rl_repo' not in sys.path:
    sys.path.insert(0, '/opt/trn_rl_repo')

import numpy as np
import ml_dtypes

bf16 = ml_dtypes.bfloat16

T = 4096
D = 1024
HL = 8          # local heads per core
HD = 64
NT = T // 128   # 32 t-tiles
KT = D // 128   # 8 contraction tiles for projections
CW = 512        # chunk width (query columns per pair-step)
NCH = T // CW   # 8 chunks
PAIRS = 4       # head pairs per core
EPS = 1.1920928955078125e-07

EXPA = 416      # cols of the [128,1024] pair-psc on Act (true Exp);
                # rest via DVE Schraudolph. Act ~600ns < 640ns PE pace.
LN2 = 0.6931471805599453
# z = bitcast_bf16(int16(psc * SCHRA + SCHRB)) ~= exp(0.125 * psc) * const
SCHRA = 0.125 * (2.0 ** 23 / LN2) / 65536.0
SCHRB = (127.0 * 2.0 ** 23 - 485000.0) / 65536.0


def build():
    from concourse import bacc, tile, mybir

    BF16 = mybir.dt.bfloat16
    F32 = mybir.dt.float32
    I16 = mybir.dt.int16
    AF = mybir.ActivationFunctionType
    ALU = mybir.AluOpType
    AX = mybir.AxisListType

    nc = bacc.Bacc()
    xT = nc.declare_dram_parameter("xT", [D, T], BF16, isOutput=False)
    wqT = nc.declare_dram_parameter("wqT", [D, 512], BF16, isOutput=False)
    wkT = nc.declare_dram_parameter("wkT", [D, 512], BF16, isOutput=False)
    wvT = nc.declare_dram_parameter("wvT", [D, 512], BF16, isOutput=False)
    woT = nc.declare_dram_parameter("woT", [512, D], BF16, isOutput=False)
    cos2 = nc.declare_dram_parameter("cos2", [T, 64], BF16, isOutput=False)
    ss = nc.declare_dram_parameter("ss", [T, 64], BF16, isOutput=False)
    ident = nc.declare_dram_parameter("ident", [128, 128], BF16, isOutput=False)
    out = nc.declare_dram_parameter("out", [T, D], F32, isOutput=True)

    with tile.TileContext(nc) as tc:
        with tc.tile_pool(name="persist", bufs=1) as persist:
            qTc = [persist.tile([128, PAIRS, CW], BF16, tag=f"qT{c}",
                                name=f"qT{c}") for c in range(NCH)]
            kT = persist.tile([128, PAIRS, T], BF16, tag="kT")
            vaug = persist.tile([128, NT, HL, 65], BF16, tag="vaug")
            wo_sb = persist.tile([128, 4, D], BF16, tag="wo_sb")
            id_sb = persist.tile([128, 128], BF16, tag="id_sb")
            eps_t = persist.tile([128, 1], F32, tag="eps_t")
            yTn = persist.tile([128, PAIRS, T], BF16, tag="yTn")

            nc.vector.memset(vaug[:, :, :, 64:65], 1.0)
            nc.vector.memset(eps_t[:], EPS)
            nc.sync.dma_start(id_sb[:], ident[:])
            nc.sync.dma_start(
                wo_sb[:], woT[:].rearrange("(k p) n -> p k n", p=128))

            # ================= Phase A: q/k/v for all t =================
            with ExitStack() as phaseA:
                wkv = phaseA.enter_context(tc.tile_pool(name="wkv", bufs=1))
                xcolp = phaseA.enter_context(
                    tc.tile_pool(name="xcolp", bufs=3))
                ascr = phaseA.enter_context(tc.tile_pool(name="ascr", bufs=3))
                asmall = phaseA.enter_context(
                    tc.tile_pool(name="asmall", bufs=3))
                ps_qkv = phaseA.enter_context(
                    tc.tile_pool(name="ps_qkv", bufs=2, space="PSUM"))
                ps_tr = phaseA.enter_context(
                    tc.tile_pool(name="ps_tr", bufs=2, space="PSUM"))

                w_sb = {}
                for name, param in (("q", wqT), ("k", wkT), ("v", wvT)):
                    w_sb[name] = wkv.tile([128, KT, 512], BF16,
                                          tag=f"w{name}", name=f"w_{name}_sb")
                    for ki in range(KT):
                        nc.sync.dma_start(
                            w_sb[name][:, ki, :],
                            param[ki * 128:(ki + 1) * 128, :])
                cos_sb = wkv.tile([128, NT, 64], BF16, tag="cos_sb")
                ss_sb = wkv.tile([128, NT, 64], BF16, tag="ss_sb")
                nc.sync.dma_start(
                    cos_sb[:], cos2[:].rearrange("(t p) d -> p t d", p=128))
                nc.sync.dma_start(
                    ss_sb[:], ss[:].rearrange("(t p) d -> p t d", p=128))

                # preload the GpSimd PartitionBroadcast library now so
                # phase C's first broadcast doesn't eat a Q7 reload
                pbsrc = asmall.tile([1, 8], F32, tag="pbsrc")
                nc.vector.memset(pbsrc[:], 1.0)
                pbdst = asmall.tile([128, 8], F32, tag="pbdst")
                nc.gpsimd.partition_broadcast(pbdst[:], pbsrc[:])

                def dma_xcol(t):
                    xcol = xcolp.tile([128, KT, 128], BF16, tag="xcol")
                    nc.sync.dma_start(
                        xcol[:],
                        xT[:, t * 128:(t + 1) * 128].rearrange(
                            "(k p) t -> p k t", p=128))
                    return xcol

                def proj(xcol, name):
                    ps = ps_qkv.tile([128, 512], F32, tag=f"p{name}",
                                     name=f"ps_{name}")
                    for ki in range(KT):
                        nc.tensor.matmul(
                            ps[:], xcol[:, ki, :], w_sb[name][:, ki, :],
                            start=(ki == 0), stop=(ki == KT - 1))
                    return ps

                def rotary_rms(t, ps_q, store, tcol):
                    """rotary + rms-normalize one projected [128,512] tile.
                    Copies + square + sqrt on Act; mults/reduce/reciprocal/
                    scale on DVE. Returns the qn tile to transpose later."""
                    ctb = cos_sb[:, t, :].unsqueeze(1).broadcast_to(
                        [128, HL, 64])
                    stb = ss_sb[:, t, :].unsqueeze(1).broadcast_to(
                        [128, HL, 64])
                    qb = ascr.tile([128, 512], BF16, tag="qb")
                    nc.scalar.copy(qb[:], ps_q[:])
                    b3 = qb[:].rearrange("p (h u d) -> p h u d", h=HL, u=2)
                    qs = ascr.tile([128, 512], BF16, tag="qs")
                    qs3 = qs[:].rearrange("p (h u d) -> p h u d", h=HL, u=2)
                    nc.scalar.copy(qs3[:, :, 0, :], b3[:, :, 1, :])
                    nc.scalar.copy(qs3[:, :, 1, :], b3[:, :, 0, :])
                    t1 = ascr.tile([128, 512], BF16, tag="t1")
                    nc.vector.tensor_tensor(
                        t1[:].rearrange("p (h d) -> p h d", h=HL),
                        qb[:].rearrange("p (h d) -> p h d", h=HL),
                        ctb, op=ALU.mult)
                    r = ascr.tile([128, 512], BF16, tag="r")
                    nc.vector.tensor_tensor(
                        r[:].rearrange("p (h d) -> p h d", h=HL),
                        qs[:].rearrange("p (h d) -> p h d", h=HL),
                        stb, op=ALU.mult)
                    nc.vector.tensor_tensor(r[:], t1[:], r[:], op=ALU.add)
                    sq = ascr.tile([128, 512], BF16, tag="sq")
                    nc.scalar.square(sq[:], r[:])
                    ms8 = asmall.tile([128, HL], F32, tag="ms8")
                    nc.vector.tensor_reduce(
                        ms8[:], sq[:].rearrange("p (h d) -> p h d", h=HL),
                        axis=AX.X, op=ALU.add)
                    rms = asmall.tile([128, HL], F32, tag="rms")
                    nc.scalar.activation(
                        rms[:], ms8[:], AF.Sqrt, scale=1.0 / HD,
                        bias=eps_t[:])
                    rinv = asmall.tile([128, HL], F32, tag="rinv")
                    nc.vector.reciprocal(rinv[:], rms[:])
                    qn = ascr.tile([128, 512], BF16, tag="qn")
                    nc.vector.tensor_tensor(
                        qn[:].rearrange("p (h d) -> p h d", h=HL),
                        r[:].rearrange("p (h d) -> p h d", h=HL),
                        rinv[:].unsqueeze(2).broadcast_to([128, HL, 64]),
                        op=ALU.mult)
                    return (qn, store, tcol)

                def emit_tr(pend, on_act):
                    qn, store, tcol = pend
                    tp = ps_tr.tile([128, 4, 128], BF16, tag="tp")
                    for cb in range(4):
                        nc.tensor.transpose(
                            tp[:, cb, :], qn[:, cb * 128:(cb + 1) * 128],
                            id_sb[:])
                    cp = nc.scalar.copy if on_act else nc.vector.tensor_copy
                    cp(store[:, :, tcol:tcol + 128], tp[:])

                pendq = []
                for t in range(NT):
                    xcol = dma_xcol(t)
                    ps_k = proj(xcol, "k")
                    ps_v = proj(xcol, "v")
                    ps_q = proj(xcol, "q")
                    nc.scalar.copy(
                        vaug[:, t, :, 0:64],
                        ps_v[:].rearrange("p (h d) -> p h d", h=HL))
                    if len(pendq) >= 4:
                        emit_tr(pendq.pop(0), on_act=False)
                    pendq.append(rotary_rms(t, ps_k, kT, t * 128))
                    if len(pendq) >= 4:
                        emit_tr(pendq.pop(0), on_act=True)
                    pendq.append(
                        rotary_rms(t, ps_q, qTc[t // 4], (t % 4) * 128))
                for i, pend in enumerate(pendq):
                    emit_tr(pend, on_act=(i % 2 == 1))
                pendq = []

            # ================= Phase C: attention =================
            with (
                tc.tile_pool(name="zp", bufs=4) as zp,
                tc.tile_pool(name="nrm", bufs=2) as nrm,
                tc.tile_pool(name="nrm1", bufs=2) as nrm1,
                tc.tile_pool(name="ps_sc", bufs=2, space="PSUM") as ps_sc,
                tc.tile_pool(name="ps_y", bufs=2, space="PSUM") as ps_y,
            ):
                # Exp table load (ATL ~2.7us) bridged by a PE junk burst so
                # the PE never idles >2us (HAM clock gate)
                dummy = nrm1.tile([128, 1], F32, tag="dummy")
                nc.scalar.activation(dummy[:], eps_t[:], AF.Exp)
                wup = ps_sc.tile([128, 2 * CW], F32, tag="psc", name="wup")
                for i in range(14):
                    nc.tensor.matmul(
                        wup[:, 0:512], kT[0:64, 0, 0:128], kT[0:64, 0, 0:512],
                        start=(i == 0), stop=(i == 13))

                def emit_attnv(e):
                    z, ya0, ya1, s, pr = e
                    nc.tensor.matmul(
                        ya0[:], vaug[:, s, 2 * pr, :], z[:, 0:CW],
                        start=(s == 0), stop=(s == NT - 1))
                    nc.tensor.matmul(
                        ya1[:], vaug[:, s, 2 * pr + 1, :], z[:, CW:2 * CW],
                        start=(s == 0), stop=(s == NT - 1))

                def make_norm(ya0, ya1, pr, ch):
                    """normalize pair (ch, pr): 7 small closures staged
                    across the next pair's s-loop (slots >= 3, i.e. after
                    this pair's final attnV has been emitted)."""
                    st = {}
                    c0 = ch * CW

                    def evac(i, ya=None):
                        yu = nrm.tile([65, CW], F32, tag=f"yu{i}",
                                      name=f"yu{i}_{ch}_{pr}")
                        nc.vector.tensor_copy(yu[:], ya[:])
                        st[f"yu{i}"] = yu

                    def bc(i):
                        dt = nrm1.tile([1, CW], F32, tag=f"dt{i}",
                                       name=f"dt{i}_{ch}_{pr}")
                        nc.vector.tensor_copy(dt[:], st[f"yu{i}"][64:65, :])
                        b = nrm1.tile([128, CW], F32, tag=f"bc{i}",
                                      name=f"bc{i}_{ch}_{pr}")
                        nc.gpsimd.partition_broadcast(b[:], dt[:])
                        st[f"bc{i}"] = b

                    def recip():
                        for i in (0, 1):
                            rc = nrm1.tile([64, CW], F32, tag=f"bcr{i}",
                                           name=f"bcr{i}_{ch}_{pr}")
                            nc.vector.reciprocal_approx_fast(
                                rc[:], st[f"bc{i}"][0:64, :])
                            st[f"r{i}"] = rc

                    def mult(i):
                        rsl = slice(64 * i, 64 * i + 64)
                        nc.vector.tensor_tensor(
                            yTn[rsl, pr, c0:c0 + CW],
                            st[f"yu{i}"][0:64, :], st[f"r{i}"][0:64, :],
                            op=ALU.mult)

                    return [lambda ya0=ya0: evac(0, ya0),
                            lambda ya1=ya1: evac(1, ya1),
                            lambda: bc(0), lambda: bc(1), recip,
                            lambda: mult(0), lambda: mult(1)]

                pend = []      # z tiles awaiting attnV (lag 3)
                norm_q = []    # previous pair's normalize closures
                for ch in range(NCH):
                    for pr in range(PAIRS):
                        ya0 = ps_y.tile([65, CW], F32, tag="ya0",
                                        name=f"ya0_{ch}_{pr}")
                        ya1 = ps_y.tile([65, CW], F32, tag="ya1",
                                        name=f"ya1_{ch}_{pr}")
                        for s in range(NT):
                            if norm_q and 3 <= s <= 9:
                                norm_q.pop(0)()
                            ssl = slice(s * 128, (s + 1) * 128)
                            psc = ps_sc.tile([128, 2 * CW], F32, tag="psc")
                            nc.tensor.matmul(
                                psc[:, 0:CW], kT[0:64, pr, ssl],
                                qTc[ch][0:64, pr, :],
                                start=True, stop=True, tile_position=(0, 0))
                            nc.tensor.matmul(
                                psc[:, CW:2 * CW], kT[64:128, pr, ssl],
                                qTc[ch][64:128, pr, :],
                                start=True, stop=True, tile_position=(64, 0))
                            z = zp.tile([128, 2 * CW], BF16, tag="z")
                            nc.scalar.activation(
                                z[:, 0:EXPA], psc[:, 0:EXPA], AF.Exp,
                                scale=0.125)
                            nc.vector.tensor_scalar(
                                z[:, EXPA:2 * CW].bitcast(I16),
                                psc[:, EXPA:2 * CW],
                                SCHRA, SCHRB, ALU.mult, ALU.add)
                            pend.append((z, ya0, ya1, s, pr))
                            if len(pend) > 3:
                                emit_attnv(pend.pop(0))
                        assert not norm_q
                        norm_q = make_norm(ya0, ya1, pr, ch)
                for e in pend:
                    emit_attnv(e)
                pend = []
                for fn in norm_q:
                    fn()
                norm_q = []

            # ===== tail: out-projection for all t =====
            with (
                tc.tile_pool(name="ps_po", bufs=4, space="PSUM") as ps_po,
                tc.tile_pool(name="ostg", bufs=4) as ostg,
            ):
                for tt in range(NT):
                    tsl = slice(tt * 128, (tt + 1) * 128)
                    for oc in range(2):
                        po = ps_po.tile([128, 512], F32, tag="po")
                        for kp in range(4):
                            nc.tensor.matmul(
                                po[:], yTn[:, kp, tsl],
                                wo_sb[:, kp, oc * 512:(oc + 1) * 512],
                                start=(kp == 0), stop=(kp == 3))
                        ost = ostg.tile([128, 512], F32, tag="ost")
                        nc.vector.tensor_copy(ost[:], po[:])
                        nc.sync.dma_start(
                            out[tsl, oc * 512:(oc + 1) * 512], ost[:])

    nc.compile()
    return nc


_CACHE = {}


def _get_nc():
    if "nc" not in _CACHE:
        _CACHE["nc"] = build()
    return _CACHE["nc"]


def _prep_inputs(x, cos, sin, wq, wk, wv, wo):
    x = np.asarray(x, dtype=np.float32)
    cos = np.asarray(cos, dtype=np.float32).reshape(T, 32)
    sin = np.asarray(sin, dtype=np.float32).reshape(T, 32)
    wq = np.asarray(wq, dtype=np.float32)
    wk = np.asarray(wk, dtype=np.float32)
    wv = np.asarray(wv, dtype=np.float32)
    wo = np.asarray(wo, dtype=np.float32)

    cos2 = np.concatenate([cos, cos], axis=1)
    ss = np.concatenate([sin, -sin], axis=1)
    ident = np.eye(128, dtype=bf16)

    in_maps = []
    for c in range(8):
        b, hg = c // 2, c % 2
        rows = slice(hg * 512, (hg + 1) * 512)
        in_maps.append({
            "xT": np.ascontiguousarray(x[b].T).astype(bf16),
            "wqT": np.ascontiguousarray(wq[rows, :].T).astype(bf16),
            "wkT": np.ascontiguousarray(wk[rows, :].T).astype(bf16),
            "wvT": np.ascontiguousarray(wv[rows, :].T).astype(bf16),
            "woT": np.ascontiguousarray(wo[:, rows].T).astype(bf16),
            "cos2": cos2.astype(bf16),
            "ss": ss.astype(bf16),
            "ident": ident,
        })
    return in_maps


def _run(in_maps, trace=False):
    from concourse.bass_utils import run_bass_kernel_spmd

    nc = _get_nc()
    res = run_bass_kernel_spmd(nc, in_maps, core_ids=list(range(8)),
                               trace=trace)
    parts = [res.results[c]["out"] for c in range(8)]
    full = np.stack([parts[2 * b] + parts[2 * b + 1] for b in range(4)])
    return full.astype(np.float32), res


def kernel(x, cos, sin, wq, wk, wv, wo):
    in_maps = _prep_inputs(x, cos, sin, wq, wk, wv, wo)
    full, _ = _run(in_maps, trace=False)
    return full
